# revision 2
# baseline (speedup 1.0000x reference)
"""Trainium2 Bass kernel: ChannelExchangeWithConv.

Reference op: lst, gui are [1, 128, 512, 512] f32.  Channels 0,2,...,126
(the ``p=2``-strided set) of out_lst are conv2(gui[:, ::2]) (a 64x64 1x1-conv
channel GEMM + bias); the same channels of out_gui are conv1(lst[:, ::2]).
Odd channels pass through unchanged.

Distribution: H (512) is sharded across 8 NeuronCores, 64 rows each — the op
is pointwise over pixels so there is no halo.  On the host each core's slice
is packed into two [128, 32768] arrays:

  ce = concat(lst[::2, rows], gui[::2, rows])   # conv inputs
  po = concat(lst[1::2, rows], gui[1::2, rows]) # passthrough

On the device a single 128x128 block-diagonal weight lhsT = diag(w1.T, w2.T)
computes BOTH 64x64 convs in one full-width matmul per 512-pixel tile
(PSUM rows 0-63 = conv1(lst_even) -> out_gui even channels, rows 64-127 =
conv2(gui_even) -> out_lst even channels).  Bias add is fused into the
PSUM->SBUF eviction on the scalar engine.  The passthrough half of the data
moves DRAM->DRAM by DMA and never touches SBUF.  The host scatters the two
per-core outputs back into the full [1, 128, 512, 512] tensors.
"""

import numpy as np

N, C, H, W = 1, 128, 512, 512
CH = C // 2          # 64 channels seen by each conv
NCORES = 8
HLOC = H // NCORES   # 64 rows of H per core
NPIX = HLOC * W      # 32768 pixels per core
P = 128              # SBUF partitions
F = 4096             # pixels per DMA chunk (2 MiB per [128, F] f32 transfer)
MM_N = 512           # moving-operand free dim per matmul (one PSUM bank, fp32 max)

_CACHE = {}
LAST_RESULTS = None  # BassKernelResults of the most recent run (test harness reads this)


def _build():
    import concourse.mybir as mybir
    import concourse.tile as tile
    from concourse import bacc

    nc = bacc.Bacc("TRN2", target_bir_lowering=False, debug=False, num_devices=NCORES)
    fp32 = mybir.dt.float32
    ce = nc.dram_tensor("ce", [P, NPIX], fp32, kind="ExternalInput").ap()
    po = nc.dram_tensor("po", [P, NPIX], fp32, kind="ExternalInput").ap()
    wt_d = nc.dram_tensor("wt", [P, P], fp32, kind="ExternalInput").ap()
    bv_d = nc.dram_tensor("bv", [P, 1], fp32, kind="ExternalInput").ap()
    co = nc.dram_tensor("co", [P, NPIX], fp32, kind="ExternalOutput").ap()
    qo = nc.dram_tensor("qo", [P, NPIX], fp32, kind="ExternalOutput").ap()

    with tile.TileContext(nc) as tc:
        with (
            tc.tile_pool(name="const", bufs=1) as const,
            tc.tile_pool(name="inp", bufs=3) as inp,
            tc.tile_pool(name="outp", bufs=3) as outp,
            tc.tile_pool(name="ps", bufs=8, space="PSUM") as pp,
        ):
            wt = const.tile([P, P], fp32)
            nc.sync.dma_start(out=wt[:], in_=wt_d)
            bt = const.tile([P, 1], fp32)
            nc.sync.dma_start(out=bt[:], in_=bv_d)
            for c in range(NPIX // F):
                sl = slice(c * F, (c + 1) * F)
                it = inp.tile([P, F], fp32)
                nc.sync.dma_start(out=it[:], in_=ce[:, sl])
                ot = outp.tile([P, F], fp32)
                for j in range(F // MM_N):
                    jsl = slice(j * MM_N, (j + 1) * MM_N)
                    ps = pp.tile([P, MM_N], fp32)
                    nc.tensor.matmul(ps[:], wt[:], it[:, jsl], start=True, stop=True)
                    nc.scalar.activation(
                        ot[:, jsl], ps[:], mybir.ActivationFunctionType.Identity,
                        bias=bt[:],
                    )
                nc.sync.dma_start(out=co[:, sl], in_=ot[:])
                # passthrough channels: DRAM -> DRAM, never touches SBUF
                nc.sync.dma_start(out=qo[:, sl], in_=po[:, sl])
    nc.compile()
    return nc


def kernel(lst, gui, w1, b1, w2, b2, p):
    global LAST_RESULTS
    from concourse.bass_utils import run_bass_kernel_spmd

    assert int(np.asarray(p)) == 2, "kernel is specialized for p=2"
    lst = np.ascontiguousarray(np.asarray(lst, dtype=np.float32))
    gui = np.ascontiguousarray(np.asarray(gui, dtype=np.float32))
    w1 = np.asarray(w1, dtype=np.float32)
    b1 = np.asarray(b1, dtype=np.float32)
    w2 = np.asarray(w2, dtype=np.float32)
    b2 = np.asarray(b2, dtype=np.float32)

    if "nc" not in _CACHE:
        _CACHE["nc"] = _build()
    nc = _CACHE["nc"]

    # lhsT for out = lhsT.T @ rhs: rows 0-63 of out = conv1 over rhs partitions
    # 0-63 (lst even channels), rows 64-127 = conv2 over partitions 64-127.
    wt = np.zeros((P, P), dtype=np.float32)
    wt[:CH, :CH] = w1.T
    wt[CH:, CH:] = w2.T
    bv = np.concatenate([b1, b2]).reshape(P, 1).astype(np.float32)

    l = lst[0]  # [C, H, W]
    g = gui[0]
    in_maps = []
    for i in range(NCORES):
        rows = slice(HLOC * i, HLOC * (i + 1))
        ce = np.concatenate([l[0::2, rows], g[0::2, rows]], axis=0).reshape(P, NPIX)
        po = np.concatenate([l[1::2, rows], g[1::2, rows]], axis=0).reshape(P, NPIX)
        in_maps.append({"ce": ce, "po": po, "wt": wt, "bv": bv})

    res = run_bass_kernel_spmd(nc, in_maps, list(range(NCORES)))
    LAST_RESULTS = res

    out_lst = np.empty_like(lst)
    out_gui = np.empty_like(gui)
    for i in range(NCORES):
        rows = slice(HLOC * i, HLOC * (i + 1))
        co = res.results[i]["co"].reshape(P, HLOC, W)
        qo = res.results[i]["qo"].reshape(P, HLOC, W)
        out_gui[0, 0::2, rows] = co[:CH]
        out_lst[0, 0::2, rows] = co[CH:]
        out_lst[0, 1::2, rows] = qo[:CH]
        out_gui[0, 1::2, rows] = qo[CH:]
    return (out_lst, out_gui)


# revision 4
# speedup vs baseline: 1.0005x; 1.0005x over previous
"""Trainium2 Bass kernel: ChannelExchangeWithConv.

Reference op: lst, gui are [1, 128, 512, 512] f32.  Channels 0,2,...,126
(the ``p=2``-strided set) of out_lst are conv2(gui[:, ::2]) (a 64x64 1x1-conv
channel GEMM + bias); the same channels of out_gui are conv1(lst[:, ::2]).
Odd channels pass through unchanged.

Distribution: H (512) is sharded across 8 NeuronCores, 64 rows each — the op
is pointwise over pixels so there is no halo.  On the host each core's slice
is packed into two [128, 32768] arrays:

  ce = concat(lst[::2, rows], gui[::2, rows])   # conv inputs
  po = concat(lst[1::2, rows], gui[1::2, rows]) # passthrough

On the device a single 128x128 block-diagonal weight lhsT = diag(w1.T, w2.T)
computes BOTH 64x64 convs in one full-width matmul per 512-pixel tile
(PSUM rows 0-63 = conv1(lst_even) -> out_gui even channels, rows 64-127 =
conv2(gui_even) -> out_lst even channels).  Bias add is fused into the
PSUM->SBUF eviction on the scalar engine.  The passthrough half of the data
moves DRAM->DRAM by DMA and never touches SBUF.  The host scatters the two
per-core outputs back into the full [1, 128, 512, 512] tensors.
"""

import numpy as np

N, C, H, W = 1, 128, 512, 512
CH = C // 2          # 64 channels seen by each conv
NCORES = 8
HLOC = H // NCORES   # 64 rows of H per core
NPIX = HLOC * W      # 32768 pixels per core
P = 128              # SBUF partitions
F = 4096             # pixels per DMA chunk (2 MiB per [128, F] f32 transfer)
MM_N = 512           # moving-operand free dim per matmul (one PSUM bank, fp32 max)

_CACHE = {}
LAST_RESULTS = None  # BassKernelResults of the most recent run (test harness reads this)


def _build():
    import concourse.mybir as mybir
    import concourse.tile as tile
    from concourse import bacc

    nc = bacc.Bacc("TRN2", target_bir_lowering=False, debug=False, num_devices=NCORES)
    fp32 = mybir.dt.float32
    ce = nc.dram_tensor("ce", [P, NPIX], fp32, kind="ExternalInput").ap()
    po = nc.dram_tensor("po", [P, NPIX], fp32, kind="ExternalInput").ap()
    wt_d = nc.dram_tensor("wt", [P, P], fp32, kind="ExternalInput").ap()
    bv_d = nc.dram_tensor("bv", [P, 1], fp32, kind="ExternalInput").ap()
    co = nc.dram_tensor("co", [P, NPIX], fp32, kind="ExternalOutput").ap()
    qo = nc.dram_tensor("qo", [P, NPIX], fp32, kind="ExternalOutput").ap()

    with tile.TileContext(nc) as tc:
        with (
            tc.tile_pool(name="const", bufs=1) as const,
            tc.tile_pool(name="inp", bufs=4) as inp,
            tc.tile_pool(name="outp", bufs=4) as outp,
            tc.tile_pool(name="ps", bufs=8, space="PSUM") as pp,
        ):
            wt = const.tile([P, P], fp32)
            nc.sync.dma_start(out=wt[:], in_=wt_d)
            bt = const.tile([P, 1], fp32)
            nc.sync.dma_start(out=bt[:], in_=bv_d)
            for c in range(NPIX // F):
                sl = slice(c * F, (c + 1) * F)
                it = inp.tile([P, F], fp32)
                nc.sync.dma_start(out=it[:], in_=ce[:, sl])
                ot = outp.tile([P, F], fp32)
                half = F // 2
                for j in range(F // MM_N):
                    jsl = slice(j * MM_N, (j + 1) * MM_N)
                    ps = pp.tile([P, MM_N], fp32)
                    nc.tensor.matmul(ps[:], wt[:], it[:, jsl], start=True, stop=True)
                    nc.scalar.activation(
                        ot[:, jsl], ps[:], mybir.ActivationFunctionType.Identity,
                        bias=bt[:],
                    )
                    # store each half as soon as its evictions are done
                    if (j + 1) * MM_N == half:
                        nc.sync.dma_start(
                            out=co[:, c * F:c * F + half], in_=ot[:, :half]
                        )
                nc.sync.dma_start(
                    out=co[:, c * F + half:(c + 1) * F], in_=ot[:, half:]
                )
                # passthrough channels: DRAM -> DRAM, never touches SBUF
                nc.sync.dma_start(out=qo[:, sl], in_=po[:, sl])
    nc.compile()
    return nc


def kernel(lst, gui, w1, b1, w2, b2, p):
    global LAST_RESULTS
    from concourse.bass_utils import run_bass_kernel_spmd

    assert int(np.asarray(p)) == 2, "kernel is specialized for p=2"
    lst = np.ascontiguousarray(np.asarray(lst, dtype=np.float32))
    gui = np.ascontiguousarray(np.asarray(gui, dtype=np.float32))
    w1 = np.asarray(w1, dtype=np.float32)
    b1 = np.asarray(b1, dtype=np.float32)
    w2 = np.asarray(w2, dtype=np.float32)
    b2 = np.asarray(b2, dtype=np.float32)

    if "nc" not in _CACHE:
        _CACHE["nc"] = _build()
    nc = _CACHE["nc"]

    # lhsT for out = lhsT.T @ rhs: rows 0-63 of out = conv1 over rhs partitions
    # 0-63 (lst even channels), rows 64-127 = conv2 over partitions 64-127.
    wt = np.zeros((P, P), dtype=np.float32)
    wt[:CH, :CH] = w1.T
    wt[CH:, CH:] = w2.T
    bv = np.concatenate([b1, b2]).reshape(P, 1).astype(np.float32)

    l = lst[0]  # [C, H, W]
    g = gui[0]
    in_maps = []
    for i in range(NCORES):
        rows = slice(HLOC * i, HLOC * (i + 1))
        ce = np.concatenate([l[0::2, rows], g[0::2, rows]], axis=0).reshape(P, NPIX)
        po = np.concatenate([l[1::2, rows], g[1::2, rows]], axis=0).reshape(P, NPIX)
        in_maps.append({"ce": ce, "po": po, "wt": wt, "bv": bv})

    res = run_bass_kernel_spmd(nc, in_maps, list(range(NCORES)))
    LAST_RESULTS = res

    out_lst = np.empty_like(lst)
    out_gui = np.empty_like(gui)
    for i in range(NCORES):
        rows = slice(HLOC * i, HLOC * (i + 1))
        co = res.results[i]["co"].reshape(P, HLOC, W)
        qo = res.results[i]["qo"].reshape(P, HLOC, W)
        out_gui[0, 0::2, rows] = co[:CH]
        out_lst[0, 0::2, rows] = co[CH:]
        out_lst[0, 1::2, rows] = qo[:CH]
        out_gui[0, 1::2, rows] = qo[CH:]
    return (out_lst, out_gui)


# revision 24
# speedup vs baseline: 1.1241x; 1.1236x over previous
"""Trainium2 Bass kernel: ChannelExchangeWithConv.

Reference op: lst, gui are [1, 128, 512, 512] f32.  Channels 0,2,...,126
(the ``p=2``-strided set) of out_lst are conv2(gui[:, ::2]) (a 64x64 1x1-conv
channel GEMM + bias); the same channels of out_gui are conv1(lst[:, ::2]).
Odd channels pass through unchanged.

Distribution: H (512) is sharded across 8 NeuronCores, 64 rows each — the op
is pointwise over pixels so there is no halo.  On the host each core's slice
is packed into two [128, 32768] arrays:

  ce = concat(lst[::2, rows], gui[::2, rows])   # conv inputs
  po = concat(lst[1::2, rows], gui[1::2, rows]) # passthrough

On the device a single 128x128 block-diagonal weight lhsT = diag(w1.T, w2.T)
computes BOTH 64x64 convs in one full-width matmul per 512-pixel tile
(PSUM rows 0-63 = conv1(lst_even) -> out_gui even channels, rows 64-127 =
conv2(gui_even) -> out_lst even channels).  Bias add is fused into the
PSUM->SBUF eviction on the scalar engine.  The passthrough half of the data
moves DRAM->DRAM by DMA and never touches SBUF.  The host scatters the two
per-core outputs back into the full [1, 128, 512, 512] tensors.
"""

import numpy as np

N, C, H, W = 1, 128, 512, 512
CH = C // 2          # 64 channels seen by each conv
NCORES = 8
HLOC = H // NCORES   # 64 rows of H per core
NPIX = HLOC * W      # 32768 pixels per core
P = 128              # SBUF partitions
F = 4096             # pixels per DMA chunk (2 MiB per [128, F] f32 transfer)
MM_N = 512           # moving-operand free dim per matmul (one PSUM bank, fp32 max)

_CACHE = {}
LAST_RESULTS = None  # BassKernelResults of the most recent run (test harness reads this)


def _build():
    import concourse.mybir as mybir
    import concourse.tile as tile
    from concourse import bacc
    from concourse.tile_rust import add_dep_helper

    nc = bacc.Bacc("TRN2", target_bir_lowering=False, debug=False, num_devices=NCORES)
    fp32 = mybir.dt.float32
    ce = nc.dram_tensor("ce", [P, NPIX], fp32, kind="ExternalInput").ap()
    po = nc.dram_tensor("po", [P, NPIX], fp32, kind="ExternalInput").ap()
    wt_d = nc.dram_tensor("wt", [P, P], fp32, kind="ExternalInput").ap()
    bv_d = nc.dram_tensor("bv", [P, 1], fp32, kind="ExternalInput").ap()
    co = nc.dram_tensor("co", [P, NPIX], fp32, kind="ExternalOutput").ap()
    qo = nc.dram_tensor("qo", [P, NPIX], fp32, kind="ExternalOutput").ap()

    with tile.TileContext(nc) as tc:
        with (
            tc.tile_pool(name="const", bufs=1) as const,
            tc.tile_pool(name="inp", bufs=4) as inp,
            tc.tile_pool(name="outp", bufs=4) as outp,
            tc.tile_pool(name="ps", bufs=8, space="PSUM") as pp,
        ):
            # consts first: their DMAs take the first sem-lane slots, so the
            # PE/ACT waits on them clear in ~1us instead of inheriting
            # multi-MB loads' completion via shared lanes.
            wt = const.tile([P, P], fp32)
            nc.sync.dma_start(out=wt[:], in_=wt_d)
            bt = const.tile([P, 1], fp32)
            nc.sync.dma_start(out=bt[:], in_=bv_d)
            # tapered chunks: small first chunk -> compute starts sooner;
            # small last chunk -> shorter store tail.
            sizes = [F // 2] + [F] * (NPIX // F - 1) + [F // 2]
            assert sum(sizes) == NPIX
            off = 0
            loads = []
            for c, sz in enumerate(sizes):
                sl = slice(off, off + sz)
                it = inp.tile([P, F], fp32, tag="it")
                ld = nc.sync.dma_start(out=it[:, :sz], in_=ce[:, sl])
                loads.append(ld)
                # passthrough channels: DRAM -> DRAM, never touches SBUF.
                # Issued from GpSimd (SWDGE): separate DMASW sem lanes and a
                # third issuer.  Paced one chunk behind the conv loads via an
                # explicit dep — unthrottled, the pre-queued d2d megabytes
                # starve the latency-critical loads at the SDMA round-robin
                # (no queue priority on trn2), delaying the whole pipeline.
                dd = nc.gpsimd.dma_start(out=qo[:, sl], in_=po[:, sl])
                add_dep_helper(
                    dd.ins, loads[c].ins, True, "pace d2d behind conv loads"
                )
                ot = outp.tile([P, F], fp32, tag="ot")
                half = sz // 2
                for j in range(sz // MM_N):
                    jsl = slice(j * MM_N, (j + 1) * MM_N)
                    ps = pp.tile([P, MM_N], fp32)
                    nc.tensor.matmul(ps[:], wt[:], it[:, jsl], start=True, stop=True)
                    nc.scalar.activation(
                        ot[:, jsl], ps[:], mybir.ActivationFunctionType.Identity,
                        bias=bt[:],
                    )
                    # store each half as soon as its evictions are done
                    if (j + 1) * MM_N == half:
                        nc.sync.dma_start(
                            out=co[:, off:off + half], in_=ot[:, :half]
                        )
                nc.sync.dma_start(
                    out=co[:, off + half:off + sz], in_=ot[:, half:sz]
                )
                off += sz
    nc.compile()
    return nc


def kernel(lst, gui, w1, b1, w2, b2, p):
    global LAST_RESULTS
    from concourse.bass_utils import run_bass_kernel_spmd

    assert int(np.asarray(p)) == 2, "kernel is specialized for p=2"
    lst = np.ascontiguousarray(np.asarray(lst, dtype=np.float32))
    gui = np.ascontiguousarray(np.asarray(gui, dtype=np.float32))
    w1 = np.asarray(w1, dtype=np.float32)
    b1 = np.asarray(b1, dtype=np.float32)
    w2 = np.asarray(w2, dtype=np.float32)
    b2 = np.asarray(b2, dtype=np.float32)

    if "nc" not in _CACHE:
        _CACHE["nc"] = _build()
    nc = _CACHE["nc"]

    # lhsT for out = lhsT.T @ rhs: rows 0-63 of out = conv1 over rhs partitions
    # 0-63 (lst even channels), rows 64-127 = conv2 over partitions 64-127.
    wt = np.zeros((P, P), dtype=np.float32)
    wt[:CH, :CH] = w1.T
    wt[CH:, CH:] = w2.T
    bv = np.concatenate([b1, b2]).reshape(P, 1).astype(np.float32)

    l = lst[0]  # [C, H, W]
    g = gui[0]
    in_maps = []
    for i in range(NCORES):
        rows = slice(HLOC * i, HLOC * (i + 1))
        ce = np.concatenate([l[0::2, rows], g[0::2, rows]], axis=0).reshape(P, NPIX)
        po = np.concatenate([l[1::2, rows], g[1::2, rows]], axis=0).reshape(P, NPIX)
        in_maps.append({"ce": ce, "po": po, "wt": wt, "bv": bv})

    try:
        res = run_bass_kernel_spmd(nc, in_maps, list(range(NCORES)))
    except ModuleNotFoundError:
        # BASS_TRACE was set but this image lacks the axon NTFF hook module;
        # rerun without tracing.
        import os

        os.environ["BASS_NEVER_TRACE"] = "1"
        res = run_bass_kernel_spmd(nc, in_maps, list(range(NCORES)))
    LAST_RESULTS = res

    out_lst = np.empty_like(lst)
    out_gui = np.empty_like(gui)
    for i in range(NCORES):
        rows = slice(HLOC * i, HLOC * (i + 1))
        co = res.results[i]["co"].reshape(P, HLOC, W)
        qo = res.results[i]["qo"].reshape(P, HLOC, W)
        out_gui[0, 0::2, rows] = co[:CH]
        out_lst[0, 0::2, rows] = co[CH:]
        out_lst[0, 1::2, rows] = qo[:CH]
        out_gui[0, 1::2, rows] = qo[CH:]
    return (out_lst, out_gui)
